# revision 34
# baseline (speedup 1.0000x reference)
"""Sparse attention (template/search) Trainium2 kernel.

Model (per batch b):
  qkv = x @ qkv_w.T                  -> split to q, k, v heads (12 heads, hd=64)
  template tokens   [0, 256)  attend to template keys only
  search   tokens [256, 1280) attend to all 1280 keys
  out = softmax(q k^T / 8) v   per head, concat heads, @ proj_w.T + proj_b

Sharding: data-parallel over batch, one batch per NeuronCore (8 cores).
No collectives needed.

Host-side layout prep (not on the device critical path): x, qkv_w, proj_w
are transposed and cast to bf16 on the host, with the softmax scale folded
into the q rows of qkv_w. No on-chip transposes at all.

Layout per core:
  - xT [C, NTOK] / wT [C, 3C] / pwT [C, C], all bf16 feature-major.
  - q,k computed feature-major: qk[f, tok] = wT[:, f].T @ xT.
  - v computed token-major, augmented per head as [1 | 63 zeros | v]: 128 wide.
  - scores computed TRANSPOSED: S.T[tk, tq] = K_h @ Q_h.T, so softmax exp is
    elementwise on [tk partitions, tq free] and no transpose of P is needed.
    The two heads of a pair sit on PE row groups 0-63 / 64-127 (64x128 array
    tiles, concurrent when adjacent in the queue).
  - AV: out[128, tq] = v_aug.T @ P.T accumulated over tk tiles; row 0 is the
    softmax denominator (from the ones column), rows 64:128 are O.T.
  - normalize off the ACT queue: DVE copy PSUM->SBUF, gpsimd
    partition_broadcast, DVE approx reciprocal, DVE multiply -> ot_all bf16.
  - proj: out[tok, c] = ot_all.T @ pwT accumulated over c_mid, + bias, bf16
    out upcast on host.

Search is processed in TWO tq passes per head pair (cj = tq columns
[256,768) then [768,1280)).  Per (pass, tk): one packed S pair -> one
[128,1024] exp covering both heads (halves ACT instruction count; ACT has a
~293ns fixed cost per instruction) -> both heads' AV.  AV emission trails S
by one tk so the in-order PE queue never head-of-line blocks on an exp
result; filler work (next pair's q/k chunks, v tiles, early proj) drains in
the remaining PE slots.  PSUM budget: st 2x[128,1024] (4 banks) + ot
2x[128,512] (2) + filler 2x[128,512] (2) = 8 banks exactly.

Pair 5's passes use early proj as filler (its cj0 normalize enables proj of
token tiles 2-5 before cj1 finishes), shrinking the PE-only tail.
"""

import numpy as np

import concourse.bacc as bacc
import concourse.mybir as mybir
import concourse.tile as tile
from concourse.masks import make_identity

P = 128
NTOK = 1280
C = 768
H = 12
HD = 64
NT = 256          # template tokens  [0, NT)
TT = NTOK // P    # 10 token tiles
CT = C // P       # 6 channel tiles
SCALE = HD ** -0.5

F32 = mybir.dt.float32
BF16 = mybir.dt.bfloat16
EXP = mybir.ActivationFunctionType.Exp
MULT = mybir.AluOpType.mult
ADD = mybir.AluOpType.add


def build_nc():
    from contextlib import ExitStack

    nc = bacc.Bacc("TRN2", target_bir_lowering=False, debug=False, num_devices=8)
    xT_ext = nc.dram_tensor("xT", [C, NTOK], BF16, kind="ExternalInput")
    wT_ext = nc.dram_tensor("wT", [C, 3 * C], BF16, kind="ExternalInput")
    pwT_ext = nc.dram_tensor("pwT", [C, C], BF16, kind="ExternalInput")
    pb_ext = nc.dram_tensor("proj_b", [1, C], F32, kind="ExternalInput")
    out_ext = nc.dram_tensor("out", [NTOK, C], BF16, kind="ExternalOutput")

    with tile.TileContext(nc) as tc, ExitStack() as ctx:
        const = ctx.enter_context(tc.tile_pool(name="const", bufs=1))
        ps_mm = ctx.enter_context(tc.tile_pool(name="ps_mm", bufs=2, space="PSUM"))
        ps_st = ctx.enter_context(tc.tile_pool(name="ps_st", bufs=2, space="PSUM"))
        ps_ot = ctx.enter_context(tc.tile_pool(name="ps_ot", bufs=2, space="PSUM"))
        big = ctx.enter_context(tc.tile_pool(name="big", bufs=1))

        ident = const.tile([P, P], F32)
        make_identity(nc, ident)
        # HAM warmup: keep the PE busy during the initial input-DMA wait so
        # its clock gate opens (1.2 -> 2.4 GHz) before the real qkv stream
        # begins.  ident.T == ident, and writing it back makes the chain live
        # (not DCE-able) and orders warmup before first real use.
        warm_ps = ps_mm.tile([P, 512], F32, tag="mm")
        for i in range(16):
            nc.tensor.transpose(warm_ps[:, :P], ident[:], ident[:])
        nc.vector.tensor_copy(ident[:], warm_ps[:, :P])
        bias_bc = const.tile([P, C], F32)
        bias_row = const.tile([1, C], F32)
        nc.sync.dma_start(bias_row[:], pb_ext.ap())
        nc.gpsimd.partition_broadcast(bias_bc[:], bias_row[0:1, :])

        xT = big.tile([P, CT, NTOK], BF16)     # x.T  (feature-major x)
        wT = big.tile([P, CT, 3 * C], BF16)    # qkv_w.T (q cols pre-scaled)
        pwT = big.tile([P, CT, C], BF16)       # proj_w.T

        # Merged 3D-AP DMAs (one instruction per group — each ~610ns of
        # sync-queue issue time, so fewer is faster) in first-need order:
        # pair-0's q/k weight tiles (ft0, ft6) and x tokens [0,512) gate the
        # first qk chunks; then v weights (template), the rest of x,
        # remaining q/k weights, proj weights.
        wsrc = wT_ext.ap().rearrange("(a p) f -> p a f", p=P)
        xsrc = xT_ext.ap().rearrange("(a p) n -> p a n", p=P)
        psrc = pwT_ext.ap().rearrange("(a p) f -> p a f", p=P)
        nc.sync.dma_start(wT[:, :, 0:P], wsrc[:, :, 0:P])            # q ft0
        nc.sync.dma_start(wT[:, :, 6 * P:7 * P],
                          wsrc[:, :, 6 * P:7 * P])                   # k ft6
        nc.sync.dma_start(xT[:, :, 0:512], xsrc[:, :, 0:512])
        nc.sync.dma_start(wT[:, :, 2 * C:], wsrc[:, :, 2 * C:])      # v
        nc.sync.dma_start(xT[:, :, 512:], xsrc[:, :, 512:])
        nc.sync.dma_start(wT[:, :, P:6 * P], wsrc[:, :, P:6 * P])    # q ft1-5
        nc.sync.dma_start(wT[:, :, 7 * P:2 * C],
                          wsrc[:, :, 7 * P:2 * C])                   # k ft7-11
        nc.sync.dma_start(pwT[:, :, :], psrc[:, :, :])

        big2 = ctx.enter_context(tc.tile_pool(name="big2", bufs=1))
        qk = big2.tile([P, 2 * CT, NTOK], BF16)     # [q (scaled) | k] feature-major
        v_sb = big2.tile([P, TT, H, P], BF16)  # [1 | 63 zeros | v]: denom row 0, O rows 64:128
        ot_all = big2.tile([P, CT, NTOK], BF16)     # attention out, feature-major
        out_sb = big2.tile([P, TT, C], BF16)

        # v_aug layout per head: col 0 = ones (softmax denominator row),
        # cols 1:64 left unwritten (they only feed PSUM rows 1:63, which
        # nothing reads; the 6.5us gpsimd memset that used to zero them
        # delayed make_identity and kept the PE warmup from starting until
        # t=13.6us), cols 64:128 = v
        nc.gpsimd.memset(v_sb[:, :, :, 0:1], 1.0)

        # ---- qkv projection (emitted interleaved with attention below) ----
        def emit_qk_chunk(ft, c0, cw):
            """qk[f, tok] = qkv_w @ x.T rows [0, 1536) for one (ftile, chunk)."""
            ps = ps_mm.tile([P, 512], F32, tag="mm", name=f"qkp{ft}_{c0}")
            for ct in range(CT):
                nc.tensor.matmul(
                    ps[:, :cw],
                    wT[:, ct, ft * P:(ft + 1) * P],
                    xT[:, ct, c0:c0 + cw],
                    start=(ct == 0), stop=(ct == CT - 1),
                )
            nc.vector.tensor_copy(qk[:, ft, c0:c0 + cw], ps[:, :cw])

        # v token-major: v[tok, f] = x @ qkv_w.T cols [1536, 2304)
        def emit_v_chunk(tt, half):
            c0, cw, h0, nh = ((0, 512, 0, 8), (512, 256, 8, 4))[half]
            ps = ps_mm.tile([P, 512], F32, tag="mm", name=f"vp{tt}_{half}")
            for ct in range(CT):
                nc.tensor.matmul(
                    ps[:, :cw],
                    xT[:, ct, tt * P:(tt + 1) * P],
                    wT[:, ct, 2 * C + c0:2 * C + c0 + cw],
                    start=(ct == 0), stop=(ct == CT - 1),
                )
            nc.vector.tensor_copy(
                v_sb[:, tt, h0:h0 + nh, 64:128],
                ps[:, :cw].rearrange("p (h e) -> p h e", e=HD),
            )

        # ---- output projection ----
        def emit_proj(tt):
            for c0, cw in ((0, 512), (512, 256)):
                ps = ps_mm.tile([P, 512], F32, tag="mm", name=f"prj{tt}_{c0}")
                for ct in range(CT):
                    nc.tensor.matmul(
                        ps[:, :cw],
                        ot_all[:, ct, tt * P:(tt + 1) * P],
                        pwT[:, ct, c0:c0 + cw],
                        start=(ct == 0), stop=(ct == CT - 1),
                    )
                nc.vector.tensor_tensor(
                    out_sb[:, tt, c0:c0 + cw], ps[:, :cw],
                    bias_bc[:, c0:c0 + cw], ADD,
                )
            nc.sync.dma_start(out_ext.ap()[tt * P:(tt + 1) * P, :],
                              out_sb[:, tt, :])

        def emit_filler(kind, arg):
            if kind == "qk":
                emit_qk_chunk(*arg)
            elif kind == "v":
                emit_v_chunk(*arg)
            else:
                emit_proj(arg)

        def qk_pair_chunks(p):
            # k chunks first (search pass 0 reads all k tiles), then q
            return ([(6 + p, c0, cw) for c0, cw in
                     ((0, 512), (512, 512), (1024, 256))]
                    + [(p, c0, cw) for c0, cw in
                       ((0, 512), (512, 512), (1024, 256))])

        # pair 0's q/k, and v token tiles 0..4, before attention starts
        # (pair 0's passes stream v tiles 5..9 as filler, one unit per
        # iteration, arriving just ahead of their AV use)
        for ft, c0, cw in qk_pair_chunks(0):
            emit_qk_chunk(ft, c0, cw)
        for tt in (0, 1, 2, 3, 4):
            emit_v_chunk(tt, 0)
            emit_v_chunk(tt, 1)

        # ---- attention ----
        pts = ctx.enter_context(tc.tile_pool(name="pts", bufs=4))
        dn = ctx.enter_context(tc.tile_pool(name="dn", bufs=2))
        rbp = ctx.enter_context(tc.tile_pool(name="rbp", bufs=2))

        def qh(h, c0, cw):
            b = (h % 2) * 64
            return qk[b:b + 64, h // 2, c0:c0 + cw]

        def kh(h, tk):
            b = (h % 2) * 64
            return qk[b:b + 64, 6 + h // 2, tk * P:(tk + 1) * P]

        def normalize(h, ot_ps, c0, cw):
            """ot_ps: [128, cw] psum (row 0 = denominators, rows 64:128 = O.T
            for tq cols [c0, c0+cw)). Normalize and write to ot_all."""
            b = (h % 2) * 64
            den = dn.tile([P, 512], F32, tag="dn")
            nc.vector.tensor_copy(den[:, :cw], ot_ps[:, :cw])
            rb = rbp.tile([P, 512], F32, tag="rb")
            nc.gpsimd.partition_broadcast(rb[:, :cw], den[0:1, :cw])
            # approx reciprocal (~18 bits, plenty for bf16 outputs)
            nc.vector.reciprocal_approx_fast(rb[:, :cw], rb[:, :cw])
            nc.vector.tensor_tensor(
                ot_all[b:b + 64, h // 2, c0:c0 + cw],
                den[64:128, :cw], rb[64:128, :cw], MULT,
            )

        def emit_av(pair, ots, pts_, tk, cj):
            pt = pts_[tk]
            for i, h in enumerate(pair):
                nc.tensor.matmul(
                    ots[h][:], v_sb[:, tk, h, :],
                    pt[:, i * 512:(i + 1) * 512],
                    start=(tk == 0), stop=(tk == TT - 1),
                )

        def emit_template(pair):
            """Template block: queries [0,256) attend keys [0,256).
            Both heads share one st tile and one [128,1024] exp."""
            st = ps_st.tile([P, 1024], F32, tag="st", name=f"tst{pair[0]}")
            for tj in range(2):
                for i, h in enumerate(pair):
                    nc.tensor.matmul(
                        st[:, i * 512 + tj * NT:i * 512 + (tj + 1) * NT],
                        kh(h, tj), qh(h, 0, NT), start=True, stop=True,
                    )
            pt = pts.tile([P, 1024], BF16, tag="pt", name=f"tpt{pair[0]}")
            nc.scalar.activation(pt[:], st[:], EXP)
            for i, h in enumerate(pair):
                to = ps_ot.tile([P, 512], F32, tag="ot", name=f"tot{h}")
                for tj in range(2):
                    nc.tensor.matmul(
                        to[:, :NT], v_sb[:, tj, h, :],
                        pt[:, i * 512 + tj * NT:i * 512 + (tj + 1) * NT],
                        start=(tj == 0), stop=(tj == 1),
                    )
                normalize(h, to, 0, NT)

        for hp in range(6):
            pair = (2 * hp, 2 * hp + 1)
            # PE filler for this pair's two search passes: next pair's q/k
            # chunks; pair 0 also streams v tiles 5..9 just-in-time; pair 5
            # uses early proj (template token tiles during pass 0, search
            # tiles 2-5 during pass 1 once pass 0's normalize has run).
            if hp < 5:
                pending = [("qk", a) for a in qk_pair_chunks(hp + 1)]
            else:
                pending = [("proj", 0), ("proj", 1)]
            if hp == 0:
                pending = [("v", (tt, half)) for tt in range(5, TT)
                           for half in (0, 1)] + pending
            emit_template(pair)
            for cj in range(2):
                if hp == 5 and cj == 1:
                    pending += [("proj", tt) for tt in (2, 3, 4, 5)]
                ots = {h: ps_ot.tile([P, 512], F32, tag="ot",
                                     name=f"ot_s{h}_{cj}") for h in pair}
                pts_ = {}
                for tk in range(TT):
                    # S^T for both heads into one 2-bank st tile; heads run
                    # concurrently on the 64-row PE array tiles (and so must
                    # write different psum banks: 512-col head offsets).
                    st = ps_st.tile([P, 1024], F32, tag="st",
                                    name=f"st{pair[0]}_{cj}_{tk}")
                    for i, h in enumerate(pair):
                        nc.tensor.matmul(
                            st[:, i * 512:(i + 1) * 512], kh(h, tk),
                            qh(h, NT + cj * 512, 512), start=True, stop=True)
                    pt = pts_[tk] = pts.tile([P, 1024], BF16, tag="pt",
                                             name=f"pt{pair[0]}_{cj}_{tk}")
                    nc.scalar.activation(pt[:], st[:], EXP)
                    # AV trails by one tk so the in-order PE queue never
                    # blocks on an exp result; fillers drain behind it.
                    if tk > 0:
                        emit_av(pair, ots, pts_, tk - 1, cj)
                        pts_.pop(tk - 1)
                    if pending and (tk % 2 == 0 or hp == 0):
                        emit_filler(*pending.pop(0))
                emit_av(pair, ots, pts_, TT - 1, cj)
                for h in pair:
                    normalize(h, ots[h], NT + cj * 512, 512)
            while pending:
                emit_filler(*pending.pop(0))

        # remaining output projection (search token tiles 6..9)
        for tt in (6, 7, 8, 9):
            emit_proj(tt)

    nc.compile()
    return nc


_NC = None


def _get_nc():
    global _NC
    if _NC is None:
        _NC = build_nc()
    return _NC


def prepare_in_maps(x, qkv_w, proj_w, proj_b):
    """Host-side layout prep: transpose + bf16 cast + q-scale folding."""
    import ml_dtypes

    BF = ml_dtypes.bfloat16
    x = np.asarray(x, dtype=np.float32)
    qkv_w = np.asarray(qkv_w, dtype=np.float32)
    proj_w = np.asarray(proj_w, dtype=np.float32)
    proj_b = np.ascontiguousarray(
        np.asarray(proj_b, dtype=np.float32)).reshape(1, C)

    w_scaled = qkv_w.copy()
    w_scaled[:C] *= SCALE                      # fold softmax scale into q
    wT = np.ascontiguousarray(w_scaled.T).astype(BF)      # [C, 3C]
    pwT = np.ascontiguousarray(proj_w.T).astype(BF)       # [C, C]
    return [
        {
            "xT": np.ascontiguousarray(x[i].T).astype(BF),  # [C, NTOK]
            "wT": wT,
            "pwT": pwT,
            "proj_b": proj_b,
        }
        for i in range(8)
    ]


def kernel(x, qkv_w, proj_w, proj_b, **_ignored):
    from concourse.bass_utils import run_bass_kernel_spmd

    nc = _get_nc()
    in_maps = prepare_in_maps(x, qkv_w, proj_w, proj_b)
    res = run_bass_kernel_spmd(nc, in_maps, list(range(8)))
    return np.stack(
        [res.results[i]["out"].astype(np.float32) for i in range(8)])


if __name__ == "__main__":
    rng = np.random.default_rng(0)
    ins = {
        "x": rng.standard_normal((8, NTOK, C), dtype=np.float32),
        "qkv_w": rng.standard_normal((3 * C, C), dtype=np.float32) * 0.02,
        "proj_w": rng.standard_normal((C, C), dtype=np.float32) * 0.02,
        "proj_b": np.zeros(C, dtype=np.float32),
    }
    out = kernel(**ins)
    print("out", out.shape, out.dtype)


# revision 37
# speedup vs baseline: 1.1670x; 1.1670x over previous
"""Sparse attention (template/search) Trainium2 kernel.

Model (per batch b):
  qkv = x @ qkv_w.T                  -> split to q, k, v heads (12 heads, hd=64)
  template tokens   [0, 256)  attend to template keys only
  search   tokens [256, 1280) attend to all 1280 keys
  out = softmax(q k^T / 8) v   per head, concat heads, @ proj_w.T + proj_b

Sharding: data-parallel over batch, one batch per NeuronCore (8 cores).
No collectives needed.

Host-side layout prep (not on the device critical path): x, qkv_w, proj_w
are transposed and cast to bf16 on the host, with the softmax scale folded
into the q rows of qkv_w. No on-chip transposes at all.

Layout per core:
  - xT [C, NTOK] / wT [C, 3C] / pwT [C, C], all bf16 feature-major.
  - q,k computed feature-major: qk[f, tok] = wT[:, f].T @ xT.
  - v computed token-major, augmented per head as [1 | 63 zeros | v]: 128 wide.
  - scores computed TRANSPOSED: S.T[tk, tq] = K_h @ Q_h.T, so softmax exp is
    elementwise on [tk partitions, tq free] and no transpose of P is needed.
    The two heads of a pair sit on PE row groups 0-63 / 64-127 (64x128 array
    tiles, concurrent when adjacent in the queue).
  - AV: out[128, tq] = v_aug.T @ P.T accumulated over tk tiles; row 0 is the
    softmax denominator (from the ones column), rows 64:128 are O.T.
  - normalize off the ACT queue: DVE copy PSUM->SBUF, gpsimd
    partition_broadcast, DVE approx reciprocal, DVE multiply -> ot_all bf16.
  - proj: out[tok, c] = ot_all.T @ pwT accumulated over c_mid, + bias, bf16
    out upcast on host.

Search is processed in TWO tq passes per head pair (cj = tq columns
[256,768) then [768,1280)).  Per (pass, tk): one packed S pair -> one
[128,1024] exp covering both heads (halves ACT instruction count; ACT has a
~293ns fixed cost per instruction) -> both heads' AV.  AV emission trails S
by one tk so the in-order PE queue never head-of-line blocks on an exp
result; filler work (next pair's q/k chunks, v tiles, early proj) drains in
the remaining PE slots.  PSUM budget: st 2x[128,1024] (4 banks) + ot
2x[128,512] (2) + filler 2x[128,512] (2) = 8 banks exactly.

Pair 5's passes use early proj as filler (its cj0 normalize enables proj of
token tiles 2-5 before cj1 finishes), shrinking the PE-only tail.
"""

import numpy as np

import concourse.bacc as bacc
import concourse.mybir as mybir
import concourse.tile as tile
from concourse.masks import make_identity

P = 128
NTOK = 1280
C = 768
H = 12
HD = 64
NT = 256          # template tokens  [0, NT)
TT = NTOK // P    # 10 token tiles
CT = C // P       # 6 channel tiles
SCALE = HD ** -0.5

F32 = mybir.dt.float32
BF16 = mybir.dt.bfloat16
EXP = mybir.ActivationFunctionType.Exp
MULT = mybir.AluOpType.mult
ADD = mybir.AluOpType.add


def build_nc():
    from contextlib import ExitStack

    nc = bacc.Bacc("TRN2", target_bir_lowering=False, debug=False, num_devices=8)
    xT_ext = nc.dram_tensor("xT", [C, NTOK], BF16, kind="ExternalInput")
    wT_ext = nc.dram_tensor("wT", [C, 3 * C], BF16, kind="ExternalInput")
    pwT_ext = nc.dram_tensor("pwT", [C, C], BF16, kind="ExternalInput")
    pb_ext = nc.dram_tensor("proj_b", [1, C], F32, kind="ExternalInput")
    out_ext = nc.dram_tensor("out", [NTOK, C], BF16, kind="ExternalOutput")

    with tile.TileContext(nc) as tc, ExitStack() as ctx:
        const = ctx.enter_context(tc.tile_pool(name="const", bufs=1))
        ps_mm = ctx.enter_context(tc.tile_pool(name="ps_mm", bufs=2, space="PSUM"))
        ps_st = ctx.enter_context(tc.tile_pool(name="ps_st", bufs=2, space="PSUM"))
        ps_ot = ctx.enter_context(tc.tile_pool(name="ps_ot", bufs=2, space="PSUM"))
        big = ctx.enter_context(tc.tile_pool(name="big", bufs=1))

        ident = const.tile([P, P], F32)
        make_identity(nc, ident)
        # HAM warmup: keep the PE busy during the initial input-DMA wait so
        # its clock gate opens (1.2 -> 2.4 GHz) before the real qkv stream
        # begins.  ident.T == ident, and writing it back makes the chain live
        # (not DCE-able) and orders warmup before first real use.
        warm_ps = ps_mm.tile([P, 512], F32, tag="mm")
        for i in range(16):
            nc.tensor.transpose(warm_ps[:, :P], ident[:], ident[:])
        nc.vector.tensor_copy(ident[:], warm_ps[:, :P])
        bias_bc = const.tile([P, C], F32)
        bias_row = const.tile([1, C], F32)
        nc.sync.dma_start(bias_row[:], pb_ext.ap())
        nc.gpsimd.partition_broadcast(bias_bc[:], bias_row[0:1, :])

        xT = big.tile([P, CT, NTOK], BF16)     # x.T  (feature-major x)
        wT = big.tile([P, CT, 3 * C], BF16)    # qkv_w.T (q cols pre-scaled)
        pwT = big.tile([P, CT, C], BF16)       # proj_w.T

        # DMA order = startup critical path: q/k weights, then x tokens
        # [0, 512) (enough for the first qk chunks + template), then the
        # rest of x, v weights, proj weights.
        for ct in range(CT):
            nc.sync.dma_start(wT[:, ct, :2 * C],
                              wT_ext.ap()[ct * P:(ct + 1) * P, :2 * C])
        for ct in range(CT):
            nc.sync.dma_start(xT[:, ct, :512],
                              xT_ext.ap()[ct * P:(ct + 1) * P, :512])
        for ct in range(CT):
            nc.sync.dma_start(xT[:, ct, 512:],
                              xT_ext.ap()[ct * P:(ct + 1) * P, 512:])
        for ct in range(CT):
            nc.sync.dma_start(wT[:, ct, 2 * C:],
                              wT_ext.ap()[ct * P:(ct + 1) * P, 2 * C:])
        for ct in range(CT):
            nc.sync.dma_start(pwT[:, ct, :],
                              pwT_ext.ap()[ct * P:(ct + 1) * P, :])

        big2 = ctx.enter_context(tc.tile_pool(name="big2", bufs=1))
        qk = big2.tile([P, 2 * CT, NTOK], BF16)     # [q (scaled) | k] feature-major
        v_sb = big2.tile([P, TT, H, P], BF16)  # [1 | 63 zeros | v]: denom row 0, O rows 64:128
        ot_all = big2.tile([P, CT, NTOK], BF16)     # attention out, feature-major
        out_sb = big2.tile([P, TT, C], BF16)

        # v_aug layout per head: col 0 = ones (softmax denominator row),
        # cols 1:64 left unwritten (they only feed PSUM rows 1:63, which
        # nothing reads; the 6.5us gpsimd memset that used to zero them
        # delayed make_identity and kept the PE warmup from starting until
        # t=13.6us), cols 64:128 = v
        nc.gpsimd.memset(v_sb[:, :, :, 0:1], 1.0)

        # ---- qkv projection (emitted interleaved with attention below) ----
        def emit_qk_chunk(ft, c0, cw):
            """qk[f, tok] = qkv_w @ x.T rows [0, 1536) for one (ftile, chunk)."""
            ps = ps_mm.tile([P, 512], F32, tag="mm", name=f"qkp{ft}_{c0}")
            for ct in range(CT):
                nc.tensor.matmul(
                    ps[:, :cw],
                    wT[:, ct, ft * P:(ft + 1) * P],
                    xT[:, ct, c0:c0 + cw],
                    start=(ct == 0), stop=(ct == CT - 1),
                )
            nc.vector.tensor_copy(qk[:, ft, c0:c0 + cw], ps[:, :cw])

        # v token-major: v[tok, f] = x @ qkv_w.T cols [1536, 2304)
        def emit_v_chunk(tt, half):
            c0, cw, h0, nh = ((0, 512, 0, 8), (512, 256, 8, 4))[half]
            ps = ps_mm.tile([P, 512], F32, tag="mm", name=f"vp{tt}_{half}")
            for ct in range(CT):
                nc.tensor.matmul(
                    ps[:, :cw],
                    xT[:, ct, tt * P:(tt + 1) * P],
                    wT[:, ct, 2 * C + c0:2 * C + c0 + cw],
                    start=(ct == 0), stop=(ct == CT - 1),
                )
            nc.vector.tensor_copy(
                v_sb[:, tt, h0:h0 + nh, 64:128],
                ps[:, :cw].rearrange("p (h e) -> p h e", e=HD),
            )

        # ---- output projection ----
        def emit_proj(tt):
            for c0, cw in ((0, 512), (512, 256)):
                ps = ps_mm.tile([P, 512], F32, tag="mm", name=f"prj{tt}_{c0}")
                for ct in range(CT):
                    nc.tensor.matmul(
                        ps[:, :cw],
                        ot_all[:, ct, tt * P:(tt + 1) * P],
                        pwT[:, ct, c0:c0 + cw],
                        start=(ct == 0), stop=(ct == CT - 1),
                    )
                nc.vector.tensor_tensor(
                    out_sb[:, tt, c0:c0 + cw], ps[:, :cw],
                    bias_bc[:, c0:c0 + cw], ADD,
                )
            nc.sync.dma_start(out_ext.ap()[tt * P:(tt + 1) * P, :],
                              out_sb[:, tt, :])

        def emit_filler(kind, arg):
            if kind == "qk":
                emit_qk_chunk(*arg)
            elif kind == "v":
                emit_v_chunk(*arg)
            else:
                emit_proj(arg)

        def qk_pair_chunks(p):
            # k chunks first (search pass 0 reads all k tiles), then q
            return ([(6 + p, c0, cw) for c0, cw in
                     ((0, 512), (512, 512), (1024, 256))]
                    + [(p, c0, cw) for c0, cw in
                       ((0, 512), (512, 512), (1024, 256))])

        # pair 0's q/k, and v token tiles 0..4, before attention starts
        # (pair 0's passes stream v tiles 5..9 as filler, one unit per
        # iteration, arriving just ahead of their AV use)
        for ft, c0, cw in qk_pair_chunks(0):
            emit_qk_chunk(ft, c0, cw)
        for tt in (0, 1, 2, 3, 4):
            emit_v_chunk(tt, 0)
            emit_v_chunk(tt, 1)

        # ---- attention ----
        pts = ctx.enter_context(tc.tile_pool(name="pts", bufs=4))
        dn = ctx.enter_context(tc.tile_pool(name="dn", bufs=2))
        rbp = ctx.enter_context(tc.tile_pool(name="rbp", bufs=2))

        def qh(h, c0, cw):
            b = (h % 2) * 64
            return qk[b:b + 64, h // 2, c0:c0 + cw]

        def kh(h, tk):
            b = (h % 2) * 64
            return qk[b:b + 64, 6 + h // 2, tk * P:(tk + 1) * P]

        def normalize(h, ot_ps, c0, cw):
            """ot_ps: [128, cw] psum (row 0 = denominators, rows 64:128 = O.T
            for tq cols [c0, c0+cw)). Normalize and write to ot_all."""
            b = (h % 2) * 64
            den = dn.tile([P, 512], F32, tag="dn")
            nc.vector.tensor_copy(den[:, :cw], ot_ps[:, :cw])
            rb = rbp.tile([P, 512], F32, tag="rb")
            nc.gpsimd.partition_broadcast(rb[:, :cw], den[0:1, :cw])
            # approx reciprocal (~18 bits, plenty for bf16 outputs)
            nc.vector.reciprocal_approx_fast(rb[:, :cw], rb[:, :cw])
            nc.vector.tensor_tensor(
                ot_all[b:b + 64, h // 2, c0:c0 + cw],
                den[64:128, :cw], rb[64:128, :cw], MULT,
            )

        def emit_av(pair, ots, pts_, tk, cj):
            pt = pts_[tk]
            for i, h in enumerate(pair):
                nc.tensor.matmul(
                    ots[h][:], v_sb[:, tk, h, :],
                    pt[:, i * 512:(i + 1) * 512],
                    start=(tk == 0), stop=(tk == TT - 1),
                )

        def emit_template(pair):
            """Template block: queries [0,256) attend keys [0,256).
            Both heads share one st tile and one [128,1024] exp."""
            st = ps_st.tile([P, 1024], F32, tag="st", name=f"tst{pair[0]}")
            for tj in range(2):
                for i, h in enumerate(pair):
                    nc.tensor.matmul(
                        st[:, i * 512 + tj * NT:i * 512 + (tj + 1) * NT],
                        kh(h, tj), qh(h, 0, NT), start=True, stop=True,
                    )
            pt = pts.tile([P, 1024], BF16, tag="pt", name=f"tpt{pair[0]}")
            nc.scalar.activation(pt[:], st[:], EXP)
            for i, h in enumerate(pair):
                to = ps_ot.tile([P, 512], F32, tag="ot", name=f"tot{h}")
                for tj in range(2):
                    nc.tensor.matmul(
                        to[:, :NT], v_sb[:, tj, h, :],
                        pt[:, i * 512 + tj * NT:i * 512 + (tj + 1) * NT],
                        start=(tj == 0), stop=(tj == 1),
                    )
                normalize(h, to, 0, NT)

        for hp in range(6):
            pair = (2 * hp, 2 * hp + 1)
            # PE filler for this pair's two search passes: next pair's q/k
            # chunks; pair 0 also streams v tiles 5..9 just-in-time; pair 5
            # uses early proj (template token tiles during pass 0, search
            # tiles 2-5 during pass 1 once pass 0's normalize has run).
            if hp < 5:
                pending = [("qk", a) for a in qk_pair_chunks(hp + 1)]
            else:
                pending = [("proj", 0), ("proj", 1)]
            if hp == 0:
                pending = [("v", (tt, half)) for tt in range(5, TT)
                           for half in (0, 1)] + pending
            emit_template(pair)
            for cj in range(2):
                if hp == 5 and cj == 1:
                    pending += [("proj", tt) for tt in (2, 3, 4, 5)]
                ots = {h: ps_ot.tile([P, 512], F32, tag="ot",
                                     name=f"ot_s{h}_{cj}") for h in pair}
                pts_ = {}
                for tk in range(TT):
                    # S^T for both heads into one 2-bank st tile; heads run
                    # concurrently on the 64-row PE array tiles (and so must
                    # write different psum banks: 512-col head offsets).
                    st = ps_st.tile([P, 1024], F32, tag="st",
                                    name=f"st{pair[0]}_{cj}_{tk}")
                    for i, h in enumerate(pair):
                        nc.tensor.matmul(
                            st[:, i * 512:(i + 1) * 512], kh(h, tk),
                            qh(h, NT + cj * 512, 512), start=True, stop=True)
                    pt = pts_[tk] = pts.tile([P, 1024], BF16, tag="pt",
                                             name=f"pt{pair[0]}_{cj}_{tk}")
                    nc.scalar.activation(pt[:], st[:], EXP)
                    # AV trails by one tk so the in-order PE queue never
                    # blocks on an exp result; fillers drain behind it.
                    if tk > 0:
                        emit_av(pair, ots, pts_, tk - 1, cj)
                        pts_.pop(tk - 1)
                    if pending and (tk % 2 == 0 or hp == 0):
                        emit_filler(*pending.pop(0))
                emit_av(pair, ots, pts_, TT - 1, cj)
                for h in pair:
                    normalize(h, ots[h], NT + cj * 512, 512)
            while pending:
                emit_filler(*pending.pop(0))

        # remaining output projection (search token tiles 6..9)
        for tt in (6, 7, 8, 9):
            emit_proj(tt)

    nc.compile()
    return nc


_NC = None


def _get_nc():
    global _NC
    if _NC is None:
        _NC = build_nc()
    return _NC


def prepare_in_maps(x, qkv_w, proj_w, proj_b):
    """Host-side layout prep: transpose + bf16 cast + q-scale folding."""
    import ml_dtypes

    BF = ml_dtypes.bfloat16
    x = np.asarray(x, dtype=np.float32)
    qkv_w = np.asarray(qkv_w, dtype=np.float32)
    proj_w = np.asarray(proj_w, dtype=np.float32)
    proj_b = np.ascontiguousarray(
        np.asarray(proj_b, dtype=np.float32)).reshape(1, C)

    w_scaled = qkv_w.copy()
    w_scaled[:C] *= SCALE                      # fold softmax scale into q
    wT = np.ascontiguousarray(w_scaled.T).astype(BF)      # [C, 3C]
    pwT = np.ascontiguousarray(proj_w.T).astype(BF)       # [C, C]
    return [
        {
            "xT": np.ascontiguousarray(x[i].T).astype(BF),  # [C, NTOK]
            "wT": wT,
            "pwT": pwT,
            "proj_b": proj_b,
        }
        for i in range(8)
    ]


def kernel(x, qkv_w, proj_w, proj_b, **_ignored):
    from concourse.bass_utils import run_bass_kernel_spmd

    nc = _get_nc()
    in_maps = prepare_in_maps(x, qkv_w, proj_w, proj_b)
    res = run_bass_kernel_spmd(nc, in_maps, list(range(8)))
    return np.stack(
        [res.results[i]["out"].astype(np.float32) for i in range(8)])


if __name__ == "__main__":
    rng = np.random.default_rng(0)
    ins = {
        "x": rng.standard_normal((8, NTOK, C), dtype=np.float32),
        "qkv_w": rng.standard_normal((3 * C, C), dtype=np.float32) * 0.02,
        "proj_w": rng.standard_normal((C, C), dtype=np.float32) * 0.02,
        "proj_b": np.zeros(C, dtype=np.float32),
    }
    out = kernel(**ins)
    print("out", out.shape, out.dtype)
